# revision 1
# baseline (speedup 1.0000x reference)
"""CTBG circuit kernel for Trainium2, data-parallel over batch on 8 NeuronCores.

Network (per reference):
  gpe_out = x @ (gpe_w * gpe_mask.T) + gpe_b              [B, 1536]
  gpi_in  = concat([x, gpe_out], -1)                      [B, 3072]
  gpi_out = gpi_in @ (gpi_w * gpi_mask.T) + gpi_b         [B, 1536]
  h1 = relu(gpi_out @ w1 + b1); h2 = relu(h1 @ w2 + b2)
  out = relu(h2 @ w3 + b3)                                [B, 6]

Per-core dataflow (feature-major activations, bf16 compute, f32 accumulate):
  - NO DMA-xbar transposes (transpose<->copy xbar-mode transitions serialize
    the whole DMA subsystem on trn2); every transpose runs on the
    TensorEngine via identity matmuls instead, overlapped with loads.
  - x and masks stream in as bf16 row-tiles (SWDGE cast-DMA straight to
    SBUF, no DRAM staging); weights stream as f32 rows on the scalar HWDGE
    queue, cast to resident bf16 tiles by DVE.
  - masked weights: PE-transpose each 128x128 mask block into PSUM, then
    DVE multiplies it into the resident bf16 weight tile in place.
  - matmul chain keeps activations feature-major; ScalarE drains PSUM with
    bias (+relu for the MLP) straight to bf16 tiles feeding the next layer.
  - output written as [6, 2048] f32 per core; host transposes + concats.
"""

import numpy as np

NCORES = 8
B = 16384
BS = B // NCORES          # 2048 rows per core
BT = 512                  # batch tile (matmul free dim)
NBT = BS // BT            # 4
D1 = 1536                 # gpe input dim
D2 = 1536                 # gpe output dim
D3 = 3072                 # gpi input dim
D4 = 1536                 # gpi output dim
H = 512                   # mlp hidden
A = 6                     # action dim

K1 = D1 // 128            # 12
U2 = D2 // 128            # 12
K3 = D3 // 128            # 24
V4 = D4 // 128            # 12
M5 = H // 128             # 4

_CACHE = {}


def _build():
    import concourse.bacc as bacc
    import concourse.tile as tile
    from concourse import mybir
    from concourse.masks import make_identity

    FP32 = mybir.dt.float32
    BF16 = mybir.dt.bfloat16
    Act = mybir.ActivationFunctionType

    nc = bacc.Bacc(None)

    x_d = nc.dram_tensor("x", [BS, D1], FP32, kind="ExternalInput")
    gpem_d = nc.dram_tensor("gpe_mask", [D2, D1], FP32, kind="ExternalInput")
    gpew_d = nc.dram_tensor("gpe_w", [D1, D2], FP32, kind="ExternalInput")
    gpeb_d = nc.dram_tensor("gpe_b", [D2], FP32, kind="ExternalInput")
    gpim_d = nc.dram_tensor("gpi_mask", [D4, D3], FP32, kind="ExternalInput")
    gpiw_d = nc.dram_tensor("gpi_w", [D3, D4], FP32, kind="ExternalInput")
    gpib_d = nc.dram_tensor("gpi_b", [D4], FP32, kind="ExternalInput")
    w1_d = nc.dram_tensor("w1", [D4, H], FP32, kind="ExternalInput")
    b1_d = nc.dram_tensor("b1", [H], FP32, kind="ExternalInput")
    w2_d = nc.dram_tensor("w2", [H, H], FP32, kind="ExternalInput")
    b2_d = nc.dram_tensor("b2", [H], FP32, kind="ExternalInput")
    w3_d = nc.dram_tensor("w3", [H, A], FP32, kind="ExternalInput")
    b3_d = nc.dram_tensor("b3", [A], FP32, kind="ExternalInput")
    o_d = nc.dram_tensor("out", [A, BS], FP32, kind="ExternalOutput")

    with tile.TileContext(nc) as tc:
        with (
            tc.tile_pool(name="wpool", bufs=1) as wp,        # persistent weights
            tc.tile_pool(name="wfpool", bufs=2) as wfp,      # transient f32 weight half-rows
            tc.tile_pool(name="mpool", bufs=3) as mp,        # transient mask row-tiles
            tc.tile_pool(name="xrpool", bufs=2) as xrp,      # transient x row-tiles
            tc.tile_pool(name="xpool", bufs=2) as xp,        # xT double-buffered
            tc.tile_pool(name="apool", bufs=1) as ap,        # activations
            tc.tile_pool(name="opool", bufs=1) as op,        # output staging
            tc.tile_pool(name="pspool", bufs=3, space="PSUM") as psp,
            tc.tile_pool(name="pstpool", bufs=3, space="PSUM") as pstp,
            tc.tile_pool(name="ps5pool", bufs=2, space="PSUM") as ps5p,
        ):
            # ---------------- SWDGE (gpsimd): bf16 cast loads ---------------
            # order = consumption order: x tile0 rows, gpe mask, gpi mask,
            # remaining x rows
            gpem_rows = []
            for u0 in range(U2):
                t = mp.tile([128, D1], BF16, tag="mrow")
                nc.gpsimd.dma_start(out=t[:, :],
                                    in_=gpem_d[u0 * 128:(u0 + 1) * 128, :])
                gpem_rows.append(t)

            xrow0 = []
            for r in range(BT // 128):
                t = xrp.tile([128, D1], BF16, tag="xr")
                nc.gpsimd.dma_start(out=t[:, :], in_=x_d[r * 128:(r + 1) * 128, :])
                xrow0.append(t)

            gpim_rows = []          # (v0, half) -> tile, loaded in v0-major order
            for v0 in range(V4):
                for hh in range(2):
                    t = mp.tile([128, D3 // 2], BF16, tag="mrow")
                    nc.gpsimd.dma_start(
                        out=t[:, :],
                        in_=gpim_d[v0 * 128:(v0 + 1) * 128,
                                   hh * (D3 // 2):(hh + 1) * (D3 // 2)])
                    gpim_rows.append(t)

            xrow_rest = []
            for t_i in range(1, NBT):
                rows = []
                for r in range(BT // 128):
                    g = t_i * (BT // 128) + r
                    t = xrp.tile([128, D1], BF16, tag="xr")
                    nc.gpsimd.dma_start(out=t[:, :],
                                        in_=x_d[g * 128:(g + 1) * 128, :])
                    rows.append(t)
                xrow_rest.append(rows)

            # ---------------- scalar HWDGE: biases + f32 weight halves ------
            ident = wp.tile([128, 128], FP32, tag="ident")
            make_identity(nc, ident[:, :])
            identb = wp.tile([128, 128], BF16, tag="identb")
            make_identity(nc, identb[:, :])

            def load_bias(b_dram, n, tag):
                nat = wp.tile([max(n, 1), 128], FP32, tag=f"{tag}_nat")
                nc.sync.dma_start(out=nat[:, :],
                                    in_=b_dram.rearrange("(c p) -> c p", p=128))
                ps = pstp.tile([128, max(n, 1)], FP32, tag="pst")
                nc.tensor.transpose(ps[:, :], nat[:, :], ident[0:n, 0:n])
                sb = wp.tile([128, max(n, 1)], FP32, tag=tag)
                nc.vector.tensor_copy(sb[:, :], ps[:, :])
                return sb

            gpeb_sb = load_bias(gpeb_d, U2, "gpeb")
            gpib_sb = load_bias(gpib_d, V4, "gpib")
            b1_sb = load_bias(b1_d, M5, "b1sb")
            b2_sb = load_bias(b2_d, M5, "b2sb")
            b3_sb = wp.tile([A, 1], FP32, tag="b3sb")
            nc.sync.dma_start(out=b3_sb[:, :],
                                in_=b3_d.rearrange("(a one) -> a one", one=1))

            def load_w_bf(w_dram, n, width, tag, halves=2):
                """f32 rows on scalar HWDGE (in `halves` column chunks) ->
                DVE cast into a resident bf16 tile."""
                tiles = []
                hw = width // halves
                for k in range(n):
                    t = wp.tile([128, width], BF16, tag=f"{tag}{k}")
                    for hh in range(halves):
                        wf = wfp.tile([128, hw], FP32, tag="wf")
                        nc.sync.dma_start(
                            out=wf[:, 0:hw],
                            in_=w_dram[k * 128:(k + 1) * 128,
                                       hh * hw:(hh + 1) * hw])
                        nc.vector.tensor_copy(t[:, hh * hw:(hh + 1) * hw],
                                              wf[:, 0:hw])
                    tiles.append(t)
                return tiles

            wgpe = load_w_bf(gpew_d, K1, D2, "wgpe")

            # ---------------- PE transposes -------------------------------
            def prep_xT(rows):
                """x row-tiles [128b, D1] -> xT chunk tiles [128i, BT]."""
                tiles = []
                for c in range(K1):
                    t = xp.tile([128, BT], BF16, tag=f"xT{c}")
                    tiles.append(t)
                for r, xrow in enumerate(rows):
                    for c in range(K1):
                        ps = pstp.tile([128, 128], BF16, tag="pst")
                        nc.tensor.transpose(ps[:, :],
                                            xrow[:, c * 128:(c + 1) * 128],
                                            identb[:, :])
                        nc.scalar.activation(
                            tiles[c][:, r * 128:(r + 1) * 128], ps[:, :],
                            mybir.ActivationFunctionType.Copy)
                return tiles

            def prep_mask(rows_for, n_out, n_k, wtiles):
                """PE-transpose mask blocks, DVE-multiply into weight tiles."""
                for o0 in range(n_out):
                    row, col0 = rows_for(o0)
                    # row covers mask[o0*128:(o0+1)*128, col0:col0+ncols]
                    ncols = row.shape[-1]
                    for cc in range(ncols // 128):
                        c = col0 // 128 + cc
                        ps = pstp.tile([128, 128], BF16, tag="pst")
                        nc.tensor.transpose(ps[:, :],
                                            row[:, cc * 128:(cc + 1) * 128],
                                            identb[:, :])
                        nc.vector.tensor_mul(
                            wtiles[c][:, o0 * 128:(o0 + 1) * 128],
                            wtiles[c][:, o0 * 128:(o0 + 1) * 128],
                            ps[:, :])

            prep_mask(lambda u0: (gpem_rows[u0], 0), U2, K1, wgpe)
            xT = prep_xT(xrow0)

            # gpi + mlp weights stream while L1 runs; emitted after the gpe
            # prep so the DVE FIFO (casts) can't head-of-line-block it
            wgpi = load_w_bf(gpiw_d, K3, D4, "wgpi")
            w1s = w2s = w3s = None

            # ---------------- main loop over batch tiles -------------------
            for t_i in range(NBT):
                # L1: gpe_out[u,b] = sum_k mw_gpe[k,u] * xT[k,b]   (+bias)
                gpe_out = []
                for u in range(U2):
                    ps = psp.tile([128, BT], FP32, tag="ps")
                    for k in range(K1):
                        nc.tensor.matmul(ps[:, :],
                                         wgpe[k][:, u * 128:(u + 1) * 128],
                                         xT[k][:, :],
                                         start=(k == 0), stop=(k == K1 - 1))
                    got = ap.tile([128, BT], BF16, tag=f"gpe_out{u}")
                    nc.scalar.activation(got[:, :], ps[:, :], Act.Identity,
                                         bias=gpeb_sb[:, u:u + 1])
                    gpe_out.append(got)
                    if t_i == 0:
                        # gpi masked-weight prep interleaved with L1 so the
                        # PE transposes and DVE muls overlap L1's matmuls
                        for hh in range(2):
                            row = gpim_rows[2 * u + hh]
                            for cc in range(K3 // 2):
                                c = hh * (K3 // 2) + cc
                                pst = pstp.tile([128, 128], BF16, tag="pst")
                                nc.tensor.transpose(
                                    pst[:, :],
                                    row[:, cc * 128:(cc + 1) * 128],
                                    identb[:, :])
                                nc.vector.tensor_mul(
                                    wgpi[c][:, u * 128:(u + 1) * 128],
                                    wgpi[c][:, u * 128:(u + 1) * 128],
                                    pst[:, :])

                if t_i == 0:
                    w1s = load_w_bf(w1_d, V4, H, "w1_", halves=1)
                    w2s = load_w_bf(w2_d, M5, H, "w2_", halves=1)
                    w3s = load_w_bf(w3_d, M5, A, "w3_", halves=1)
                    xT_next = prep_xT(xrow_rest[0])
                elif t_i + 1 < NBT:
                    xT_next = prep_xT(xrow_rest[t_i])
                else:
                    xT_next = None

                # L2: gpi_out[v,b] = sum_k mw_gpi[k,v] * gpi_in[k,b] (+bias)
                gpi_out = []
                for v in range(V4):
                    ps = psp.tile([128, BT], FP32, tag="ps")
                    for k in range(K3):
                        rhs = xT[k] if k < K1 else gpe_out[k - K1]
                        nc.tensor.matmul(ps[:, :],
                                         wgpi[k][:, v * 128:(v + 1) * 128],
                                         rhs[:, :],
                                         start=(k == 0), stop=(k == K3 - 1))
                    gio = ap.tile([128, BT], BF16, tag=f"gpi_out{v}")
                    nc.scalar.activation(gio[:, :], ps[:, :], Act.Identity,
                                         bias=gpib_sb[:, v:v + 1])
                    gpi_out.append(gio)

                # L3: h1 = relu(gpi_out @ w1 + b1)
                h1 = []
                for m in range(M5):
                    ps = psp.tile([128, BT], FP32, tag="ps")
                    for k in range(V4):
                        nc.tensor.matmul(ps[:, :],
                                         w1s[k][:, m * 128:(m + 1) * 128],
                                         gpi_out[k][:, :],
                                         start=(k == 0), stop=(k == V4 - 1))
                    hm = ap.tile([128, BT], BF16, tag=f"h1_{m}")
                    nc.scalar.activation(hm[:, :], ps[:, :], Act.Relu,
                                         bias=b1_sb[:, m:m + 1])
                    h1.append(hm)

                # L4: h2 = relu(h1 @ w2 + b2)
                h2 = []
                for m in range(M5):
                    ps = psp.tile([128, BT], FP32, tag="ps")
                    for k in range(M5):
                        nc.tensor.matmul(ps[:, :],
                                         w2s[k][:, m * 128:(m + 1) * 128],
                                         h1[k][:, :],
                                         start=(k == 0), stop=(k == M5 - 1))
                    hm = ap.tile([128, BT], BF16, tag=f"h2_{m}")
                    nc.scalar.activation(hm[:, :], ps[:, :], Act.Relu,
                                         bias=b2_sb[:, m:m + 1])
                    h2.append(hm)

                # L5: out = relu(h2 @ w3 + b3), [6, BT] f32
                ps5 = ps5p.tile([A, BT], FP32, tag="ps5")
                for k in range(M5):
                    nc.tensor.matmul(ps5[:, :], w3s[k][:, :], h2[k][:, :],
                                     start=(k == 0), stop=(k == M5 - 1))
                osb = op.tile([A, BT], FP32, tag="osb")
                nc.scalar.activation(osb[:, :], ps5[:, :], Act.Relu,
                                     bias=b3_sb[:, 0:1])
                nc.sync.dma_start(out=o_d[:, t_i * BT:(t_i + 1) * BT],
                                    in_=osb[:, :])

                if xT_next is not None:
                    xT = xT_next

    nc.finalize()
    return nc


def _get_nc():
    if "nc" not in _CACHE:
        _CACHE["nc"] = _build()
    return _CACHE["nc"]


def _run(inputs, trace=False):
    from concourse.bass_utils import run_bass_kernel_spmd

    nc = _get_nc()
    shared = {k: np.ascontiguousarray(v, dtype=np.float32)
              for k, v in inputs.items() if k != "x"}
    x = np.ascontiguousarray(inputs["x"], dtype=np.float32)
    in_maps = [dict(shared, x=x[c * BS:(c + 1) * BS]) for c in range(NCORES)]
    res = run_bass_kernel_spmd(nc, in_maps, list(range(NCORES)), trace=trace)
    out = np.concatenate(
        [np.asarray(res.results[c]["out"]).T for c in range(NCORES)], axis=0)
    return out.astype(np.float32), res


def kernel(**inputs):
    out, _ = _run(inputs, trace=False)
    return out



# revision 9
# speedup vs baseline: 1.1076x; 1.1076x over previous
"""CTBG circuit kernel for Trainium2, data-parallel over batch on 8 NeuronCores.

Network (per reference):
  gpe_out = x @ (gpe_w * gpe_mask.T) + gpe_b              [B, 1536]
  gpi_in  = concat([x, gpe_out], -1)                      [B, 3072]
  gpi_out = gpi_in @ (gpi_w * gpi_mask.T) + gpi_b         [B, 1536]
  h1 = relu(gpi_out @ w1 + b1); h2 = relu(h1 @ w2 + b2)
  out = relu(h2 @ w3 + b3)                                [B, 6]

Per-core dataflow (feature-major activations, bf16 compute, f32 accumulate):
  - Two concurrent DMA streams so the load phase runs near the HBM
    roofline instead of serializing on one queue:
      * SWDGE (gpsimd) casts x and all weights f32->bf16 in multi-MB
        transfers straight into the matmul-ready [128, k, n] layouts.
      * HWDGE (sync) streams the masks as raw f32 row-tiles.
  - Masks are PE-transposed (f32, identity matmul) into PSUM; DVE
    multiplies them into the resident bf16 weight tiles in place.
  - x row-chunks are PE-transposed to feature-major xT tiles (bf16).
  - Matmul chain keeps activations feature-major; ScalarE drains PSUM
    with bias (+relu for the MLP) straight to bf16 tiles feeding the
    next layer.
  - Output written as [6, 2048] f32 per core; host transposes + concats.
"""

import numpy as np

NCORES = 8
B = 16384
BS = B // NCORES          # 2048 rows per core
BT = 512                  # batch tile (matmul free dim)
NBT = BS // BT            # 4
D1 = 1536                 # gpe input dim
D2 = 1536                 # gpe output dim
D3 = 3072                 # gpi input dim
D4 = 1536                 # gpi output dim
H = 512                   # mlp hidden
A = 6                     # action dim

K1 = D1 // 128            # 12
U2 = D2 // 128            # 12
K3 = D3 // 128            # 24
V4 = D4 // 128            # 12
M5 = H // 128             # 4

_CACHE = {}


def _build():
    import concourse.bacc as bacc
    import concourse.tile as tile
    from concourse import mybir
    from concourse.masks import make_identity

    FP32 = mybir.dt.float32
    BF16 = mybir.dt.bfloat16
    Act = mybir.ActivationFunctionType

    nc = bacc.Bacc(None)

    x_d = nc.dram_tensor("x", [BS, D1], FP32, kind="ExternalInput")
    gpem_d = nc.dram_tensor("gpe_mask", [D2, D1], FP32, kind="ExternalInput")
    gpew_d = nc.dram_tensor("gpe_w", [D1, D2], FP32, kind="ExternalInput")
    gpeb_d = nc.dram_tensor("gpe_b", [D2], FP32, kind="ExternalInput")
    gpim_d = nc.dram_tensor("gpi_mask", [D4, D3], FP32, kind="ExternalInput")
    gpiw_d = nc.dram_tensor("gpi_w", [D3, D4], FP32, kind="ExternalInput")
    gpib_d = nc.dram_tensor("gpi_b", [D4], FP32, kind="ExternalInput")
    w1_d = nc.dram_tensor("w1", [D4, H], FP32, kind="ExternalInput")
    b1_d = nc.dram_tensor("b1", [H], FP32, kind="ExternalInput")
    w2_d = nc.dram_tensor("w2", [H, H], FP32, kind="ExternalInput")
    b2_d = nc.dram_tensor("b2", [H], FP32, kind="ExternalInput")
    w3_d = nc.dram_tensor("w3", [H, A], FP32, kind="ExternalInput")
    b3_d = nc.dram_tensor("b3", [A], FP32, kind="ExternalInput")
    o_d = nc.dram_tensor("out", [A, BS], FP32, kind="ExternalOutput")

    with tile.TileContext(nc) as tc:
        with (
            tc.tile_pool(name="wpool", bufs=1) as wp,        # persistent weights
            tc.tile_pool(name="mpool", bufs=2) as mp,        # mask f32 row-tiles
            tc.tile_pool(name="xrpool", bufs=2) as xrp,      # x row-chunks bf16
            tc.tile_pool(name="xpool", bufs=2) as xp,        # xT double-buffered
            tc.tile_pool(name="apool", bufs=1) as ap,        # activations
            tc.tile_pool(name="hpool", bufs=1) as hp,        # mlp activations
            tc.tile_pool(name="opool", bufs=2) as op,        # output staging
            tc.tile_pool(name="pspool", bufs=3, space="PSUM") as psp,
            tc.tile_pool(name="pstpool", bufs=2, space="PSUM") as pstp,
            tc.tile_pool(name="ps5pool", bufs=1, space="PSUM") as ps5p,
        ):
            # ------------- SWDGE (gpsimd): big bf16 cast loads ------------
            # FIFO order = consumption order: x tile0, gpe weights, x rest,
            # gpi weights, mlp weights.
            xrow = {}
            for t_i in range(NBT):
                for hh in range(4):
                    t = xrp.tile([128, D1], BF16, tag="xr")
                    r0 = t_i * 4 + hh
                    nc.gpsimd.dma_start(
                        out=t[:, :],
                        in_=x_d[r0 * 128:(r0 + 1) * 128, :])
                    xrow[(t_i, hh)] = t
                if t_i == 0:
                    wgpe = wp.tile([128, K1, D2], BF16, tag="wgpe")
                    nc.gpsimd.dma_start(
                        out=wgpe[:, :, :],
                        in_=gpew_d.rearrange("(k p) n -> p k n", p=128))

            wgpi = wp.tile([128, K3, D4], BF16, tag="wgpi")
            for c in range(2):
                nc.gpsimd.dma_start(
                    out=wgpi[:, c * (K3 // 2):(c + 1) * (K3 // 2), :],
                    in_=gpiw_d[c * (D3 // 2):(c + 1) * (D3 // 2), :].rearrange(
                        "(k p) n -> p k n", p=128))

            w1s = wp.tile([128, V4, H], BF16, tag="w1s")
            nc.gpsimd.dma_start(
                out=w1s[:, :, :],
                in_=w1_d.rearrange("(k p) n -> p k n", p=128))
            w2s = wp.tile([128, M5, H], BF16, tag="w2s")
            nc.gpsimd.dma_start(
                out=w2s[:, :, :],
                in_=w2_d.rearrange("(k p) n -> p k n", p=128))
            w3s = wp.tile([128, M5, A], BF16, tag="w3s")
            nc.gpsimd.dma_start(
                out=w3s[:, :, :],
                in_=w3_d.rearrange("(k p) a -> p k a", p=128))

            # ------------- HWDGE (sync): masks (raw f32) + biases ---------
            ident = wp.tile([128, 128], FP32, tag="ident")
            make_identity(nc, ident[:, :])
            identb = wp.tile([128, 128], BF16, tag="identb")
            make_identity(nc, identb[:, :])

            def load_bias(b_dram, n, tag):
                nat = mp.tile([max(n, 1), 128], FP32, tag="bnat")
                nc.sync.dma_start(out=nat[:, :],
                                  in_=b_dram.rearrange("(c p) -> c p", p=128))
                ps = pstp.tile([128, 128], FP32, tag="pstf")
                nc.tensor.transpose(ps[0:128, 0:n], nat[:, :], ident[0:n, 0:n])
                sb = wp.tile([128, max(n, 1)], FP32, tag=tag)
                nc.vector.tensor_copy(sb[:, 0:n], ps[0:128, 0:n])
                return sb

            gpeb_sb = load_bias(gpeb_d, U2, "gpeb")
            gpib_sb = load_bias(gpib_d, V4, "gpib")
            b1_sb = load_bias(b1_d, M5, "b1sb")
            b2_sb = load_bias(b2_d, M5, "b2sb")
            b3_sb = wp.tile([A, 1], FP32, tag="b3sb")
            nc.sync.dma_start(out=b3_sb[:, :],
                              in_=b3_d.rearrange("(a one) -> a one", one=1))

            # mask row-tiles: [128, D1] f32 staged through a small ring;
            # gpi mask rows arrive as two half-row DMAs per 128 features
            def load_gpem_row(u0):
                t = mp.tile([128, D1], FP32, tag="mrow")
                nc.sync.dma_start(out=t[:, :],
                                  in_=gpem_d[u0 * 128:(u0 + 1) * 128, :])
                return t

            def load_gpim_half(v0, half):
                t = mp.tile([128, D1], FP32, tag="mrow")
                nc.sync.dma_start(
                    out=t[:, :],
                    in_=gpim_d[v0 * 128:(v0 + 1) * 128,
                               half * D1:(half + 1) * D1])
                return t

            # ------------- PE prep: mask transposes + x transposes --------
            def prep_mask_row(row, u0, c0, ncols, wtile):
                """transpose mask row [128, ncols*128] block by block and
                multiply into wtile[:, c0+c, u0-slice]."""
                for c in range(ncols):
                    ps = pstp.tile([128, 128], FP32, tag="pstf")
                    nc.tensor.transpose(ps[:, :],
                                        row[:, c * 128:(c + 1) * 128],
                                        ident[:, :])
                    nc.vector.tensor_mul(
                        wtile[:, c0 + c, u0 * 128:(u0 + 1) * 128],
                        wtile[:, c0 + c, u0 * 128:(u0 + 1) * 128],
                        ps[:, :])

            def prep_xT(t_i):
                """x rows [128, D1] -> xT tile [128, K1, BT]."""
                xt = xp.tile([128, K1, BT], BF16, tag="xT")
                for g in range(4):
                    rows = xrow[(t_i, g)]
                    for c in range(K1):
                        ps = pstp.tile([128, 128], BF16, tag="pstb")
                        nc.tensor.transpose(
                            ps[:, :], rows[:, c * 128:(c + 1) * 128],
                            identb[:, :])
                        nc.scalar.activation(
                            xt[:, c, g * 128:(g + 1) * 128], ps[:, :],
                            Act.Copy)
                return xt

            # gpe mask prep (stream 12 rows x 12 cols), then first xT
            for u0 in range(U2):
                row = load_gpem_row(u0)
                prep_mask_row(row, u0, 0, K1, wgpe)
            xT = prep_xT(0)

            # ------------- main loop over batch tiles ---------------------
            gpe_out = ap.tile([128, U2, BT], BF16, tag="gpe_out")
            gpi_out = ap.tile([128, V4, BT], BF16, tag="gpi_out")

            for t_i in range(NBT):
                # L1: gpe_out[u] = sum_k mw_gpe[k,u] @ xT[k]  (+bias)
                for u in range(U2):
                    ps = psp.tile([128, BT], FP32, tag="ps")
                    for k in range(K1):
                        nc.tensor.matmul(ps[:, :],
                                         wgpe[:, k, u * 128:(u + 1) * 128],
                                         xT[:, k, :],
                                         start=(k == 0), stop=(k == K1 - 1))
                    nc.scalar.activation(gpe_out[:, u, :], ps[:, :],
                                         Act.Identity,
                                         bias=gpeb_sb[:, u:u + 1])
                    if t_i == 0:
                        # gpi masked-weight prep interleaved with L1 so the
                        # PE transposes and DVE muls overlap L1's matmuls
                        for half in range(2):
                            row = load_gpim_half(u, half)
                            prep_mask_row(row, u, half * K1, K1, wgpi)

                xT_next = prep_xT(t_i + 1) if t_i + 1 < NBT else None

                # L2: gpi_out[v] = sum_k mw_gpi[k,v] @ gpi_in[k]  (+bias)
                for v in range(V4):
                    ps = psp.tile([128, BT], FP32, tag="ps")
                    for k in range(K3):
                        rhs = xT[:, k, :] if k < K1 else gpe_out[:, k - K1, :]
                        nc.tensor.matmul(ps[:, :],
                                         wgpi[:, k, v * 128:(v + 1) * 128],
                                         rhs,
                                         start=(k == 0), stop=(k == K3 - 1))
                    nc.scalar.activation(gpi_out[:, v, :], ps[:, :],
                                         Act.Identity,
                                         bias=gpib_sb[:, v:v + 1])

                # L3: h1 = relu(gpi_out @ w1 + b1)
                h1 = hp.tile([128, M5, BT], BF16, tag="h1")
                for m in range(M5):
                    ps = psp.tile([128, BT], FP32, tag="ps")
                    for k in range(V4):
                        nc.tensor.matmul(ps[:, :],
                                         w1s[:, k, m * 128:(m + 1) * 128],
                                         gpi_out[:, k, :],
                                         start=(k == 0), stop=(k == V4 - 1))
                    nc.scalar.activation(h1[:, m, :], ps[:, :], Act.Relu,
                                         bias=b1_sb[:, m:m + 1])

                # L4: h2 = relu(h1 @ w2 + b2)
                h2 = hp.tile([128, M5, BT], BF16, tag="h2")
                for m in range(M5):
                    ps = psp.tile([128, BT], FP32, tag="ps")
                    for k in range(M5):
                        nc.tensor.matmul(ps[:, :],
                                         w2s[:, k, m * 128:(m + 1) * 128],
                                         h1[:, k, :],
                                         start=(k == 0), stop=(k == M5 - 1))
                    nc.scalar.activation(h2[:, m, :], ps[:, :], Act.Relu,
                                         bias=b2_sb[:, m:m + 1])

                # L5: out = relu(h2 @ w3 + b3), [6, BT] f32
                ps5 = ps5p.tile([A, BT], FP32, tag="ps5")
                for k in range(M5):
                    nc.tensor.matmul(ps5[:, :], w3s[:, k, :], h2[:, k, :],
                                     start=(k == 0), stop=(k == M5 - 1))
                osb = op.tile([A, BT], FP32, tag="osb")
                nc.scalar.activation(osb[:, :], ps5[:, :], Act.Relu,
                                     bias=b3_sb[:, 0:1])
                nc.sync.dma_start(out=o_d[:, t_i * BT:(t_i + 1) * BT],
                                  in_=osb[:, :])

                if xT_next is not None:
                    xT = xT_next

    nc.finalize()
    return nc


def _get_nc():
    if "nc" not in _CACHE:
        _CACHE["nc"] = _build()
    return _CACHE["nc"]


def _run(inputs, trace=False):
    from concourse.bass_utils import run_bass_kernel_spmd

    nc = _get_nc()
    shared = {k: np.ascontiguousarray(v, dtype=np.float32)
              for k, v in inputs.items() if k != "x"}
    x = np.ascontiguousarray(inputs["x"], dtype=np.float32)
    in_maps = [dict(shared, x=x[c * BS:(c + 1) * BS]) for c in range(NCORES)]
    res = run_bass_kernel_spmd(nc, in_maps, list(range(NCORES)), trace=trace)
    out = np.concatenate(
        [np.asarray(res.results[c]["out"]).T for c in range(NCORES)], axis=0)
    return out.astype(np.float32), res


def kernel(**inputs):
    out, _ = _run(inputs, trace=False)
    return out


# revision 16
# speedup vs baseline: 1.1576x; 1.0451x over previous
"""CTBG circuit kernel for Trainium2, data-parallel over batch on 8 NeuronCores.

Network (per reference):
  gpe_out = x @ (gpe_w * gpe_mask.T) + gpe_b              [B, 1536]
  gpi_in  = concat([x, gpe_out], -1)                      [B, 3072]
  gpi_out = gpi_in @ (gpi_w * gpi_mask.T) + gpi_b         [B, 1536]
  h1 = relu(gpi_out @ w1 + b1); h2 = relu(h1 @ w2 + b2)
  out = relu(h2 @ w3 + b3)                                [B, 6]

Per-core dataflow (feature-major activations, bf16 compute, f32 accumulate):
  - Two concurrent DMA streams so the load phase runs near the HBM
    roofline instead of serializing on one queue:
      * SWDGE (gpsimd) casts x and all weights f32->bf16 in multi-MB
        transfers straight into the matmul-ready [128, k, n] layouts.
      * HWDGE (sync) streams the masks as raw f32 row-tiles.
  - Masks are PE-transposed (f32, identity matmul) into PSUM; DVE
    multiplies them into the resident bf16 weight tiles in place.
  - x row-chunks are PE-transposed to feature-major xT tiles (bf16).
  - Matmul chain keeps activations feature-major; ScalarE drains PSUM
    with bias (+relu for the MLP) straight to bf16 tiles feeding the
    next layer.
  - Output written as [6, 2048] f32 per core; host transposes + concats.
"""

import numpy as np

NCORES = 8
B = 16384
BS = B // NCORES          # 2048 rows per core
BT = 512                  # batch tile (matmul free dim)
NBT = BS // BT            # 4
D1 = 1536                 # gpe input dim
D2 = 1536                 # gpe output dim
D3 = 3072                 # gpi input dim
D4 = 1536                 # gpi output dim
H = 512                   # mlp hidden
A = 6                     # action dim

K1 = D1 // 128            # 12
U2 = D2 // 128            # 12
K3 = D3 // 128            # 24
V4 = D4 // 128            # 12
M5 = H // 128             # 4

_CACHE = {}


def _build():
    import concourse.bacc as bacc
    import concourse.tile as tile
    from concourse import mybir
    from concourse.masks import make_identity

    FP32 = mybir.dt.float32
    BF16 = mybir.dt.bfloat16
    Act = mybir.ActivationFunctionType

    nc = bacc.Bacc(None)

    x_d = nc.dram_tensor("x", [BS, D1], FP32, kind="ExternalInput")
    gpem_d = nc.dram_tensor("gpe_mask", [D2, D1], FP32, kind="ExternalInput")
    gpew_d = nc.dram_tensor("gpe_w", [D1, D2], FP32, kind="ExternalInput")
    gpeb_d = nc.dram_tensor("gpe_b", [D2], FP32, kind="ExternalInput")
    gpim_d = nc.dram_tensor("gpi_mask", [D4, D3], FP32, kind="ExternalInput")
    gpiw_d = nc.dram_tensor("gpi_w", [D3, D4], FP32, kind="ExternalInput")
    gpib_d = nc.dram_tensor("gpi_b", [D4], FP32, kind="ExternalInput")
    w1_d = nc.dram_tensor("w1", [D4, H], FP32, kind="ExternalInput")
    b1_d = nc.dram_tensor("b1", [H], FP32, kind="ExternalInput")
    w2_d = nc.dram_tensor("w2", [H, H], FP32, kind="ExternalInput")
    b2_d = nc.dram_tensor("b2", [H], FP32, kind="ExternalInput")
    w3_d = nc.dram_tensor("w3", [H, A], FP32, kind="ExternalInput")
    b3_d = nc.dram_tensor("b3", [A], FP32, kind="ExternalInput")
    o_d = nc.dram_tensor("out", [A, BS], FP32, kind="ExternalOutput")

    with tile.TileContext(nc) as tc:
        with (
            tc.tile_pool(name="wpool", bufs=1) as wp,        # persistent weights
            tc.tile_pool(name="mpool", bufs=2) as mp,        # mask row staging
            tc.tile_pool(name="xrpool", bufs=2) as xrp,      # x row-chunks bf16
            tc.tile_pool(name="xpool", bufs=1) as xp,        # xT feature-major
            tc.tile_pool(name="apool", bufs=1) as ap,        # activations
            tc.tile_pool(name="hpool", bufs=1) as hp,        # mlp activations
            tc.tile_pool(name="opool", bufs=2) as op,        # output staging
            tc.tile_pool(name="pspool", bufs=3, space="PSUM") as psp,
            tc.tile_pool(name="pstpool", bufs=2, space="PSUM") as pstp,
            tc.tile_pool(name="ps5pool", bufs=1, space="PSUM") as ps5p,
        ):
            # ------------- SWDGE (gpsimd): big bf16 cast loads ------------
            # FIFO order = consumption order; x tiles 1-3 go last so their
            # staging-ring WAR stalls cannot block the weight stream.
            def load_x_chunk(t_i, hh):
                t = xrp.tile([128, 2, D1], BF16, tag="xr")
                r0 = t_i * 4 + hh * 2
                nc.gpsimd.dma_start(
                    out=t[:, :, :],
                    in_=x_d[r0 * 128:(r0 + 2) * 128, :].rearrange(
                        "(r p) n -> p r n", p=128))
                return t

            xrow = {}
            wgpe = wp.tile([128, K1, D2], BF16, tag="wgpe")
            nc.gpsimd.dma_start(
                out=wgpe[:, :, :],
                in_=gpew_d.rearrange("(k p) n -> p k n", p=128))
            for hh in range(2):
                xrow[(0, hh)] = load_x_chunk(0, hh)

            wgpi = wp.tile([128, K3, D4], BF16, tag="wgpi")
            for c in range(2):
                nc.gpsimd.dma_start(
                    out=wgpi[:, c * (K3 // 2):(c + 1) * (K3 // 2), :],
                    in_=gpiw_d[c * (D3 // 2):(c + 1) * (D3 // 2), :].rearrange(
                        "(k p) n -> p k n", p=128))

            w1s = wp.tile([128, V4, H], BF16, tag="w1s")
            nc.gpsimd.dma_start(
                out=w1s[:, :, :],
                in_=w1_d.rearrange("(k p) n -> p k n", p=128))
            w2s = wp.tile([128, M5, H], BF16, tag="w2s")
            nc.gpsimd.dma_start(
                out=w2s[:, :, :],
                in_=w2_d.rearrange("(k p) n -> p k n", p=128))
            w3s = wp.tile([128, M5, A], BF16, tag="w3s")
            nc.gpsimd.dma_start(
                out=w3s[:, :, :],
                in_=w3_d.rearrange("(k p) a -> p k a", p=128))
            for t_i in range(1, NBT):
                for hh in range(2):
                    xrow[(t_i, hh)] = load_x_chunk(t_i, hh)

            # ------------- HWDGE (sync): masks (raw f32) + biases ---------
            ident = wp.tile([128, 128], FP32, tag="ident")
            make_identity(nc, ident[:, :])
            identb = wp.tile([128, 128], BF16, tag="identb")
            make_identity(nc, identb[:, :])

            def load_bias(b_dram, n, tag):
                nat = mp.tile([max(n, 1), 128], FP32, tag="bnat")
                nc.sync.dma_start(out=nat[:, :],
                                  in_=b_dram.rearrange("(c p) -> c p", p=128))
                ps = pstp.tile([128, 128], FP32, tag="pstf")
                nc.tensor.transpose(ps[0:128, 0:n], nat[:, :], ident[0:n, 0:n])
                sb = wp.tile([128, max(n, 1)], FP32, tag=tag)
                nc.vector.tensor_copy(sb[:, 0:n], ps[0:128, 0:n])
                return sb

            gpeb_sb = load_bias(gpeb_d, U2, "gpeb")
            gpib_sb = load_bias(gpib_d, V4, "gpib")
            b1_sb = load_bias(b1_d, M5, "b1sb")
            b2_sb = load_bias(b2_d, M5, "b2sb")
            b3_sb = wp.tile([A, 1], FP32, tag="b3sb")
            nc.sync.dma_start(out=b3_sb[:, :],
                              in_=b3_d.rearrange("(a one) -> a one", one=1))

            # mask row-tiles: raw f32 rows staged through a small ring, then
            # DVE-cast to bf16 so the PE transposes get FWL (2x faster than
            # fp32 transpose mode); gpi rows arrive as two half-row DMAs.
            def load_mask_row(dram, u0, half):
                t = mp.tile([128, D1], FP32, tag="mrow")
                nc.sync.dma_start(
                    out=t[:, :],
                    in_=dram[u0 * 128:(u0 + 1) * 128,
                             half * D1:(half + 1) * D1])
                tb = mp.tile([128, D1], BF16, tag="mrowb")
                nc.vector.tensor_copy(tb[:, :], t[:, :])
                return tb

            # ------------- PE prep: mask transposes + x transposes --------
            def prep_mask_row(row, u0, c0, ncols, wtile):
                """transpose mask row [128, ncols*128] block by block and
                multiply into wtile[:, c0+c, u0-slice]."""
                for c in range(ncols):
                    ps = pstp.tile([128, 128], BF16, tag="pstb")
                    nc.tensor.transpose(ps[:, :],
                                        row[:, c * 128:(c + 1) * 128],
                                        identb[:, :])
                    nc.vector.tensor_mul(
                        wtile[:, c0 + c, u0 * 128:(u0 + 1) * 128],
                        wtile[:, c0 + c, u0 * 128:(u0 + 1) * 128],
                        ps[:, :])

            def prep_xT(t_i, xt):
                """x chunks [128, 2, D1] -> xT tile [128, K1, BT]."""
                for hh in range(2):
                    rows = xrow[(t_i, hh)]
                    for r in range(2):
                        g = hh * 2 + r
                        for c in range(K1):
                            ps = pstp.tile([128, 128], BF16, tag="pstb")
                            nc.tensor.transpose(
                                ps[:, :], rows[:, r, c * 128:(c + 1) * 128],
                                identb[:, :])
                            nc.scalar.activation(
                                xt[:, c, g * 128:(g + 1) * 128], ps[:, :],
                                Act.Copy)

            # gpe mask prep (stream 12 rows x 12 cols), then first xT
            for u0 in range(U2):
                row = load_mask_row(gpem_d, u0, 0)
                prep_mask_row(row, u0, 0, K1, wgpe)
            xT = xp.tile([128, K1, BT], BF16, tag="xT")
            prep_xT(0, xT)

            # ------------- main loop over batch tiles ---------------------
            gpe_out = ap.tile([128, U2, BT], BF16, tag="gpe_out")
            gpi_out = ap.tile([128, V4, BT], BF16, tag="gpi_out")

            for t_i in range(NBT):
                # L1: gpe_out[u] = sum_k mw_gpe[k,u] @ xT[k]  (+bias)
                for u in range(U2):
                    ps = psp.tile([128, BT], FP32, tag="ps")
                    for k in range(K1):
                        nc.tensor.matmul(ps[:, :],
                                         wgpe[:, k, u * 128:(u + 1) * 128],
                                         xT[:, k, :],
                                         start=(k == 0), stop=(k == K1 - 1))
                    nc.scalar.activation(gpe_out[:, u, :], ps[:, :],
                                         Act.Identity,
                                         bias=gpeb_sb[:, u:u + 1])
                    if t_i == 0:
                        # gpi masked-weight prep interleaved with L1 so the
                        # PE transposes and DVE muls overlap L1's matmuls
                        for half in range(2):
                            row = load_mask_row(gpim_d, u, half)
                            prep_mask_row(row, u, half * K1, K1, wgpi)

                # L2: gpi_out[v] = sum_k mw_gpi[k,v] @ gpi_in[k]  (+bias)
                for v in range(V4):
                    ps = psp.tile([128, BT], FP32, tag="ps")
                    for k in range(K3):
                        rhs = xT[:, k, :] if k < K1 else gpe_out[:, k - K1, :]
                        nc.tensor.matmul(ps[:, :],
                                         wgpi[:, k, v * 128:(v + 1) * 128],
                                         rhs,
                                         start=(k == 0), stop=(k == K3 - 1))
                    nc.scalar.activation(gpi_out[:, v, :], ps[:, :],
                                         Act.Identity,
                                         bias=gpib_sb[:, v:v + 1])

                # next tile's xT (single buffer: xT-t frees after L2-t)
                if t_i + 1 < NBT:
                    prep_xT(t_i + 1, xT)

                # L3: h1 = relu(gpi_out @ w1 + b1)
                h1 = hp.tile([128, M5, BT], BF16, tag="h1")
                for m in range(M5):
                    ps = psp.tile([128, BT], FP32, tag="ps")
                    for k in range(V4):
                        nc.tensor.matmul(ps[:, :],
                                         w1s[:, k, m * 128:(m + 1) * 128],
                                         gpi_out[:, k, :],
                                         start=(k == 0), stop=(k == V4 - 1))
                    nc.scalar.activation(h1[:, m, :], ps[:, :], Act.Relu,
                                         bias=b1_sb[:, m:m + 1])

                # L4: h2 = relu(h1 @ w2 + b2)
                h2 = hp.tile([128, M5, BT], BF16, tag="h2")
                for m in range(M5):
                    ps = psp.tile([128, BT], FP32, tag="ps")
                    for k in range(M5):
                        nc.tensor.matmul(ps[:, :],
                                         w2s[:, k, m * 128:(m + 1) * 128],
                                         h1[:, k, :],
                                         start=(k == 0), stop=(k == M5 - 1))
                    nc.scalar.activation(h2[:, m, :], ps[:, :], Act.Relu,
                                         bias=b2_sb[:, m:m + 1])

                # L5: out = relu(h2 @ w3 + b3), [6, BT] f32
                ps5 = ps5p.tile([A, BT], FP32, tag="ps5")
                for k in range(M5):
                    nc.tensor.matmul(ps5[:, :], w3s[:, k, :], h2[:, k, :],
                                     start=(k == 0), stop=(k == M5 - 1))
                osb = op.tile([A, BT], FP32, tag="osb")
                nc.scalar.activation(osb[:, :], ps5[:, :], Act.Relu,
                                     bias=b3_sb[:, 0:1])
                nc.sync.dma_start(out=o_d[:, t_i * BT:(t_i + 1) * BT],
                                  in_=osb[:, :])

    nc.finalize()
    return nc


def _get_nc():
    if "nc" not in _CACHE:
        _CACHE["nc"] = _build()
    return _CACHE["nc"]


def _run(inputs, trace=False):
    from concourse.bass_utils import run_bass_kernel_spmd

    nc = _get_nc()
    shared = {k: np.ascontiguousarray(v, dtype=np.float32)
              for k, v in inputs.items() if k != "x"}
    x = np.ascontiguousarray(inputs["x"], dtype=np.float32)
    in_maps = [dict(shared, x=x[c * BS:(c + 1) * BS]) for c in range(NCORES)]
    res = run_bass_kernel_spmd(nc, in_maps, list(range(NCORES)), trace=trace)
    out = np.concatenate(
        [np.asarray(res.results[c]["out"]).T for c in range(NCORES)], axis=0)
    return out.astype(np.float32), res


def kernel(**inputs):
    out, _ = _run(inputs, trace=False)
    return out
